# revision 2
# baseline (speedup 1.0000x reference)
"""Trainium2 Bass kernel for MatrixGATVAE (2-layer GATv2 encoder), 8-core SPMD.

kernel(**inputs): FULL numpy inputs -> FULL [20000, 128] f32 output.
Sharding: nodes + in-edges by destination across 8 cores; weights replicated;
xl-side tables all-gathered.

Exact math transforms (numpy-verified):
- Conv1d(kernel==stride) + flatten folded into projection weights V.
- att.leaky_relu(xl[s]+xr[d]) = sum_{att>0} LRa(za) - sum_{att<0} LRa(za),
  za = |att| (xl + xr) per column, hidden space permuted so att>0 cols lead.
- softmax without segment-max (identical up to fp rounding; logits are O(1)).
- BatchNorm folded into layer-2 projection weights on device.
"""

import math
import sys

import numpy as np

sys.path.insert(0, "/opt/trn_rl_repo")

import concourse.bass as bass
import concourse.bacc as bacc
import concourse.mybir as mybir
import concourse.tile as tile
from concourse.library_config import mlp as MLP_LIB
from concourse.bass_utils import run_bass_kernel_spmd

BF16 = mybir.dt.bfloat16
F32 = mybir.dt.float32
I16 = mybir.dt.int16
I32 = mybir.dt.int32
AF = mybir.ActivationFunctionType
ALU = mybir.AluOpType

N = 20000
NCORES = 8
NLOC = N // NCORES            # 2500
NPAD = 2560
NB = NPAD // 128              # 20
KW, TPOS, COUT = 24, 32, 64
F0 = 768
H = 512
L = 128
KC0 = F0 // 128               # 6
KC1 = H // 128                # 4
BN_EPS = 1e-5
ALPHA = 0.2

_cache = {}


def _bf16(a):
    import ml_dtypes
    return np.ascontiguousarray(np.asarray(a, dtype=np.float32)).astype(ml_dtypes.bfloat16)


def _batches(tb):
    out, t = [], 0
    while t < tb:
        nt = min(4, tb - t)
        out.append((t, nt))
        t += nt
    return out


def build_program(TB, Fp1, Fp2, debug=False):
    nc = bacc.Bacc(num_devices=NCORES)

    def dp(name, shape, dtype, isOutput=False):
        return nc.declare_dram_parameter(name, shape, dtype, isOutput)

    xT = dp("xT", [128, KC0, NPAD], BF16)
    Wl = dp("Wl", [128, KC0, H], BF16)
    Wra = dp("Wra", [128, KC0, H], BF16)
    beff_l = dp("beff_l", [1, H], BF16)
    beff_ra = dp("beff_ra", [1, H], BF16)
    catt1 = dp("catt1", [128, H], BF16)
    bias1b = dp("bias1b", [128, H], BF16)
    W2l = dp("W2l", [128, KC1, L], BF16)
    W2ra = dp("W2ra", [128, KC1, L], BF16)
    b2l = dp("b2l", [1, L], BF16)
    b2ra = dp("b2ra", [1, L], BF16)
    catt2 = dp("catt2", [128, L], BF16)
    bias2b = dp("bias2b", [128, L], F32)
    gamma_c = dp("gamma_c", [128, KC1], F32)
    beta_c = dp("beta_c", [128, KC1], F32)
    iota_t = dp("iota_t", [128, 128], F32)
    maskl = dp("maskl", [128, 1], F32)
    isrc = dp("isrc", [128, NB, TB], I32)
    idst = dp("idst", [128, NB, TB], I32)
    dstrel = dp("dstrel", [128, NB, TB], F32)
    mu_out = dp("mu", [NPAD, L], F32, isOutput=True)
    if debug:
        dbg_q = dp("dbg_q", [128, 3 * NB * TB], F32, isOutput=True)
        dbg_h1 = dp("dbg_h1", [NPAD, H], F32, isOutput=True)
        dbg_xl = dp("dbg_xl", [NPAD, H], F32, isOutput=True)
        dbg_xra = dp("dbg_xra", [NPAD, H], F32, isOutput=True)
        dbg_xl2 = dp("dbg_xl2", [NPAD, L], F32, isOutput=True)
        dbg_xra2 = dp("dbg_xra2", [NPAD, L], F32, isOutput=True)
        dbg_bn = dp("dbg_bn", [128, 6 * KC1], F32, isOutput=True)
        dbg_h1t = dp("dbg_h1t", [128, NPAD], F32, isOutput=True)
        dbg_b2e = dp("dbg_b2e", [1, 2 * L], F32, isOutput=True)

    XL_loc = nc.dram_tensor("XL_loc", [NPAD, H], BF16)
    XLA_loc = nc.dram_tensor("XLA_loc", [NPAD, H], BF16)
    XL = nc.dram_tensor("XL", [N, H], BF16, addr_space="Shared")
    XLA = nc.dram_tensor("XLA", [N, H], BF16, addr_space="Shared")
    XRA1 = nc.dram_tensor("XRA1", [NPAD, H], BF16)
    H1D = nc.dram_tensor("H1D", [NPAD, H], BF16)
    H1Dc = [nc.dram_tensor(f"H1Dc{c}", [NPAD, 128], BF16) for c in range(KC1)]
    XL2_loc = nc.dram_tensor("XL2_loc", [NPAD, L], BF16)
    XLA2_loc = nc.dram_tensor("XLA2_loc", [NPAD, L], BF16)
    XL2 = nc.dram_tensor("XL2", [N, L], BF16, addr_space="Shared")
    XLA2 = nc.dram_tensor("XLA2", [N, L], BF16, addr_space="Shared")
    XRA2 = nc.dram_tensor("XRA2", [NPAD, L], BF16)
    ST_loc = nc.dram_tensor("ST_loc", [128, 2 * KC1], F32)
    ST_red = nc.dram_tensor("ST_red", [128, 2 * KC1], F32, addr_space="Shared")

    grp = [list(range(NCORES))]

    with tile.TileContext(nc) as tc:
        with (
            tc.tile_pool(name="const", bufs=1) as cpool,
            tc.tile_pool(name="work", bufs=3) as work,
            tc.tile_pool(name="gath", bufs=3) as gath,
            tc.tile_pool(name="blk", bufs=2) as blk,
            tc.tile_pool(name="acc", bufs=3, space="PSUM") as ps,
            tc.tile_pool(name="psS", bufs=2, space="PSUM") as ps_s,
            tc.tile_pool(name="psT", bufs=2, space="PSUM") as ps_st,
        ):
            def load(tag, dram):
                t = cpool.tile(dram.shape, dram.dtype, tag=tag)
                nc.sync.dma_start(t[:], dram[:])
                return t

            xT_s = load("xT", xT)
            Wl_s = load("Wl", Wl)
            Wra_s = load("Wra", Wra)
            beffl_s = load("beffl", beff_l)
            beffra_s = load("beffra", beff_ra)
            catt1_s = load("catt1", catt1)
            bias1b_s = load("bias1b", bias1b)
            W2l_s = load("W2l", W2l)
            W2ra_s = load("W2ra", W2ra)
            b2l_s = load("b2l", b2l)
            b2ra_s = load("b2ra", b2ra)
            catt2_s = load("catt2", catt2)
            bias2b_s = load("bias2b", bias2b)
            gamma_s = load("gamma", gamma_c)
            beta_s = load("beta", beta_c)
            iota_s = load("iota", iota_t)
            maskl_s = load("maskl", maskl)
            isrc_s = load("isrc", isrc)
            idst_s = load("idst", idst)
            dstrel_s = load("dstrel", dstrel)

            ones_col = cpool.tile([128, 1], BF16, tag="ones_col")
            nc.vector.memset(ones_col[:], 1.0)
            one_row = cpool.tile([1, 128], BF16, tag="one_row")
            nc.vector.memset(one_row[:], 1.0)

            # persistent strips
            Q1 = cpool.tile([128, NB, TB], F32, tag="Q1")
            Q2 = cpool.tile([128, NB, TB], F32, tag="Q2")
            AA = cpool.tile([128, NB, TB], F32, tag="AA")
            h1T = cpool.tile([128, KC1, NPAD], BF16, tag="h1T")
            W2fl = cpool.tile([128, KC1, L], BF16, tag="W2fl")
            W2fra = cpool.tile([128, KC1, L], BF16, tag="W2fra")
            b2e = cpool.tile([1, L], BF16, tag="b2e")
            b2era = cpool.tile([1, L], BF16, tag="b2era")

            # ============ layer-1 projections -> tables ============
            for b in range(NB):
                sl = slice(b * 128, (b + 1) * 128)
                pl = ps.tile([128, H], F32, tag="acc")
                pra = ps.tile([128, H], F32, tag="acc")
                for c in range(KC0):
                    lhsT = xT_s[:, c, sl]
                    nc.tensor.matmul(pl[:], lhsT, Wl_s[:, c, :], start=(c == 0), stop=False)
                    nc.tensor.matmul(pra[:], lhsT, Wra_s[:, c, :], start=(c == 0), stop=False)
                nc.tensor.matmul(pl[:], one_row[:1, :], beffl_s[:1, :], start=False, stop=True)
                nc.tensor.matmul(pra[:], one_row[:1, :], beffra_s[:1, :], start=False, stop=True)
                xl_sb = blk.tile([128, H], BF16, tag="xl")
                xla_sb = blk.tile([128, H], BF16, tag="xla")
                xra_sb = blk.tile([128, H], BF16, tag="xra")
                nc.vector.tensor_copy(xl_sb[:], pl[:])
                nc.vector.tensor_tensor(xla_sb[:], xl_sb[:], catt1_s[:], ALU.mult)
                nc.scalar.activation(xra_sb[:], pra[:], AF.Copy)
                nc.sync.dma_start(XL_loc[sl, :], xl_sb[:])
                nc.sync.dma_start(XLA_loc[sl, :], xla_sb[:])
                nc.sync.dma_start(XRA1[sl, :], xra_sb[:])

            nc.gpsimd.collective_compute(
                "AllGather", ALU.bypass, replica_groups=grp,
                ins=[XL_loc[0:NLOC, :]], outs=[XL[:, :]])
            nc.gpsimd.collective_compute(
                "AllGather", ALU.bypass, replica_groups=grp,
                ins=[XLA_loc[0:NLOC, :]], outs=[XLA[:, :]])

            # ============ generic edge stage ============
            def logits_sweep(W, Fp, TBL, TBLR, lay):
                for b in range(NB):
                    for t in range(TB):
                        ga = gath.tile([128, W], BF16, tag=f"ga{lay}")
                        gr = gath.tile([128, W], BF16, tag=f"gr{lay}")
                        nc.gpsimd.indirect_dma_start(
                            out=ga[:], out_offset=None, in_=TBL[:, :],
                            in_offset=bass.IndirectOffsetOnAxis(
                                ap=isrc_s[:, b, t:t + 1], axis=0))
                        nc.gpsimd.indirect_dma_start(
                            out=gr[:], out_offset=None, in_=TBLR[:, :],
                            in_offset=bass.IndirectOffsetOnAxis(
                                ap=idst_s[:, b, t:t + 1], axis=0))
                        z = work.tile([128, W], BF16, tag=f"z{lay}")
                        nc.vector.tensor_tensor(z[:], ga[:], gr[:], ALU.add)
                        lr1 = work.tile([128, W], BF16, tag=f"lr{lay}")
                        if Fp > 0:
                            nc.scalar.activation(
                                lr1[:, 0:Fp], z[:, 0:Fp], AF.Prelu,
                                alpha=ALPHA, accum_out=Q1[:, b, t:t + 1])
                        else:
                            nc.vector.memset(Q1[:, b, t:t + 1], 0.0)
                        if Fp < W:
                            nc.scalar.activation(
                                lr1[:, Fp:W], z[:, Fp:W], AF.Prelu,
                                alpha=ALPHA, accum_out=Q2[:, b, t:t + 1])
                        else:
                            nc.vector.memset(Q2[:, b, t:t + 1], 0.0)
                # a = exp(q1 - q2), one batch
                nc.vector.tensor_tensor(AA[:, :, :], Q1[:, :, :], Q2[:, :, :], ALU.subtract)
                nc.scalar.activation(AA[:, :, :], AA[:, :, :], AF.Exp)

            def agg_sweep(W, TBL, lay, post):
                for b in range(NB):
                    pU = ps.tile([128, W], F32, tag="acc")
                    pS = ps_s.tile([128, 1], F32, tag="pS")
                    for t in range(TB):
                        gm = gath.tile([128, W], BF16, tag=f"gm{lay}")
                        nc.gpsimd.indirect_dma_start(
                            out=gm[:], out_offset=None, in_=TBL[:, :],
                            in_offset=bass.IndirectOffsetOnAxis(
                                ap=isrc_s[:, b, t:t + 1], axis=0))
                        Sa = work.tile([128, 128], BF16, tag=f"Sa{lay}")
                        nc.vector.tensor_scalar(
                            Sa[:], iota_s[:], dstrel_s[:, b, t:t + 1],
                            AA[:, b, t:t + 1], ALU.is_equal, ALU.mult)
                        nc.tensor.matmul(
                            pU[:], Sa[:], gm[:],
                            start=(t == 0), stop=(t == TB - 1))
                        nc.tensor.matmul(
                            pS[:], Sa[:], ones_col[:],
                            start=(t == 0), stop=(t == TB - 1))
                    post(b, pU, pS)

            # ---- layer-1: logits, softmax, aggregate -> h1 ----
            logits_sweep(H, Fp1, XLA, XRA1, 1)

            st_acc = cpool.tile([128, 2 * KC1], F32, tag="st_acc")
            nc.vector.memset(st_acc[:], 0.0)

            def post1(b, pU, pS):
                s_sb = blk.tile([128, 1], F32, tag="s1")
                nc.vector.tensor_scalar(s_sb[:], pS[:], 1e-16, None, ALU.add)
                r_sb = blk.tile([128, 1], F32, tag="r1")
                nc.vector.reciprocal(r_sb[:], s_sb[:])
                h1 = blk.tile([128, H], BF16, tag="h1")
                # h1 = relu(U * r + bias1)  (bias1 then relu then mask)
                nc.vector.tensor_scalar(h1[:], pU[:], r_sb[:], None, ALU.mult)
                nc.vector.tensor_tensor(h1[:], h1[:], bias1b_s[:], ALU.add)
                nc.vector.tensor_scalar(h1[:], h1[:], 0.0, None, ALU.max)
                if b == NB - 1:
                    nc.vector.tensor_scalar(h1[:], h1[:], maskl_s[:], None, ALU.mult)
                sq = blk.tile([128, H], BF16, tag="sq")
                nc.scalar.activation(sq[:], h1[:], AF.Square)
                pmb = ps_st.tile([128, 2 * KC1], F32, tag="st")
                for c in range(KC1):
                    nc.tensor.matmul(
                        pmb[:, c:c + 1], h1[:, c * 128:(c + 1) * 128], ones_col[:],
                        start=True, stop=True)
                    nc.tensor.matmul(
                        pmb[:, KC1 + c:KC1 + c + 1], sq[:, c * 128:(c + 1) * 128], ones_col[:],
                        start=True, stop=True)
                nc.vector.tensor_tensor(st_acc[:], st_acc[:], pmb[:], ALU.add)
                nc.sync.dma_start(H1D[b * 128:(b + 1) * 128, :], h1[:])
                for c in range(KC1):
                    nc.sync.dma_start(
                        H1Dc[c][b * 128:(b + 1) * 128, :],
                        h1[:, c * 128:(c + 1) * 128])

            agg_sweep(H, XL, 1, post1)

            # ---- BN stats all-reduce + fold into layer-2 weights ----
            nc.sync.dma_start(ST_loc[:, :], st_acc[:])
            nc.gpsimd.collective_compute(
                "AllReduce", ALU.add, replica_groups=grp,
                ins=[ST_loc[:, :]], outs=[ST_red[:, :]])
            str_sb = cpool.tile([128, 2 * KC1], F32, tag="str_sb")
            nc.sync.dma_start(str_sb[:], ST_red[:, :])

            mean = cpool.tile([128, KC1], F32, tag="mean")
            var = cpool.tile([128, KC1], F32, tag="var")
            nc.vector.tensor_scalar(mean[:], str_sb[:, 0:KC1], 1.0 / N, None, ALU.mult)
            nc.vector.tensor_scalar(var[:], str_sb[:, KC1:2 * KC1], 1.0 / N, None, ALU.mult)
            m2 = cpool.tile([128, KC1], F32, tag="m2")
            nc.vector.tensor_tensor(m2[:], mean[:], mean[:], ALU.mult)
            nc.vector.tensor_tensor(var[:], var[:], m2[:], ALU.subtract)
            nc.vector.tensor_scalar(var[:], var[:], BN_EPS, None, ALU.add)
            sd = cpool.tile([128, KC1], F32, tag="sd")
            nc.scalar.activation(sd[:], var[:], AF.Sqrt)
            # one Newton step: sd = 0.5*(sd + var/sd)
            rsd = cpool.tile([128, KC1], F32, tag="rsd")
            nc.vector.reciprocal(rsd[:], sd[:])
            tmpn = cpool.tile([128, KC1], F32, tag="tmpn")
            nc.vector.tensor_tensor(tmpn[:], var[:], rsd[:], ALU.mult)
            nc.vector.tensor_tensor(sd[:], sd[:], tmpn[:], ALU.add)
            nc.vector.tensor_scalar(sd[:], sd[:], 0.5, None, ALU.mult)
            nc.vector.reciprocal(rsd[:], sd[:])
            scale = cpool.tile([128, KC1], F32, tag="scale")
            nc.vector.tensor_tensor(scale[:], gamma_s[:], rsd[:], ALU.mult)
            shift = cpool.tile([128, KC1], F32, tag="shift")
            nc.vector.tensor_tensor(shift[:], mean[:], scale[:], ALU.mult)
            nc.vector.tensor_tensor(shift[:], beta_s[:], shift[:], ALU.subtract)
            shift_bf = cpool.tile([128, KC1], BF16, tag="shift_bf")
            nc.vector.tensor_copy(shift_bf[:], shift[:])

            for c in range(KC1):
                nc.vector.tensor_scalar(
                    W2fl[:, c, :], W2l_s[:, c, :], scale[:, c:c + 1], None, ALU.mult)
                nc.vector.tensor_scalar(
                    W2fra[:, c, :], W2ra_s[:, c, :], scale[:, c:c + 1], None, ALU.mult)
            pb = ps_st.tile([1, L], F32, tag="st")
            pbra = ps_st.tile([1, L], F32, tag="st")
            for c in range(KC1):
                nc.tensor.matmul(pb[:1, :], shift_bf[:, c:c + 1], W2l_s[:, c, :],
                                 start=(c == 0), stop=False)
                nc.tensor.matmul(pbra[:1, :], shift_bf[:, c:c + 1], W2ra_s[:, c, :],
                                 start=(c == 0), stop=False)
            nc.tensor.matmul(pb[:1, :], one_row[:1, 0:1], b2l_s[:1, :], start=False, stop=True)
            nc.tensor.matmul(pbra[:1, :], one_row[:1, 0:1], b2ra_s[:1, :], start=False, stop=True)
            nc.vector.tensor_copy(b2e[:], pb[:1, :])
            nc.vector.tensor_copy(b2era[:], pbra[:1, :])

            # h1T via DMA transpose (bf16, part%16, free%128)
            for c in range(KC1):
                nc.sync.dma_start(h1T[:, c, :], H1Dc[c][:, :], transpose=True)

            # ---- layer-2 projections -> tables ----
            for b in range(NB):
                sl = slice(b * 128, (b + 1) * 128)
                p2 = ps.tile([128, L], F32, tag="acc")
                p2ra = ps.tile([128, L], F32, tag="acc")
                for c in range(KC1):
                    lhsT = h1T[:, c, sl]
                    nc.tensor.matmul(p2[:], lhsT, W2fl[:, c, :], start=(c == 0), stop=False)
                    nc.tensor.matmul(p2ra[:], lhsT, W2fra[:, c, :], start=(c == 0), stop=False)
                nc.tensor.matmul(p2[:], one_row[:1, :], b2e[:1, :], start=False, stop=True)
                nc.tensor.matmul(p2ra[:], one_row[:1, :], b2era[:1, :], start=False, stop=True)
                xl2_sb = blk.tile([128, L], BF16, tag="xl2")
                xla2_sb = blk.tile([128, L], BF16, tag="xla2")
                xra2_sb = blk.tile([128, L], BF16, tag="xra2")
                nc.vector.tensor_copy(xl2_sb[:], p2[:])
                nc.vector.tensor_tensor(xla2_sb[:], xl2_sb[:], catt2_s[:], ALU.mult)
                nc.scalar.activation(xra2_sb[:], p2ra[:], AF.Copy)
                nc.sync.dma_start(XL2_loc[sl, :], xl2_sb[:])
                nc.sync.dma_start(XLA2_loc[sl, :], xla2_sb[:])
                nc.sync.dma_start(XRA2[sl, :], xra2_sb[:])

            nc.gpsimd.collective_compute(
                "AllGather", ALU.bypass, replica_groups=grp,
                ins=[XL2_loc[0:NLOC, :]], outs=[XL2[:, :]])
            nc.gpsimd.collective_compute(
                "AllGather", ALU.bypass, replica_groups=grp,
                ins=[XLA2_loc[0:NLOC, :]], outs=[XLA2[:, :]])

            # ---- layer-2 edge stage -> mu ----
            logits_sweep(L, Fp2, XLA2, XRA2, 2)

            def post2(b, pU, pS):
                s_sb = blk.tile([128, 1], F32, tag="s2")
                nc.vector.tensor_scalar(s_sb[:], pS[:], 1e-16, None, ALU.add)
                r_sb = blk.tile([128, 1], F32, tag="r2")
                nc.vector.reciprocal(r_sb[:], s_sb[:])
                mu_sb = blk.tile([128, L], F32, tag="mu")
                nc.vector.tensor_scalar(mu_sb[:], pU[:], r_sb[:], None, ALU.mult)
                nc.vector.tensor_tensor(mu_sb[:], mu_sb[:], bias2b_s[:], ALU.add)
                nc.sync.dma_start(mu_out[b * 128:(b + 1) * 128, :], mu_sb[:])

            agg_sweep(L, XL2, 2, post2)

            if debug:
                # ---- debug dumps ----
                qd = cpool.tile([128, 3 * NB * TB], F32, tag="qd")
                nc.vector.tensor_copy(qd[:, 0:NB * TB], Q1[:, :, :])
                nc.vector.tensor_copy(qd[:, NB * TB:2 * NB * TB], Q2[:, :, :])
                nc.vector.tensor_copy(qd[:, 2 * NB * TB:3 * NB * TB], AA[:, :, :])
                nc.sync.dma_start(dbg_q[:, :], qd[:])
                for b in range(NB):
                    sl = slice(b * 128, (b + 1) * 128)
                    t1 = blk.tile([128, H], F32, tag="dbgc")
                    g1 = gath.tile([128, H], BF16, tag="dbgg")
                    nc.sync.dma_start(g1[:], H1D[sl, :])
                    nc.vector.tensor_copy(t1[:], g1[:])
                    nc.sync.dma_start(dbg_h1[sl, :], t1[:])
                    t2 = blk.tile([128, H], F32, tag="dbgc")
                    g2 = gath.tile([128, H], BF16, tag="dbgg")
                    nc.sync.dma_start(g2[:], XL_loc[sl, :])
                    nc.vector.tensor_copy(t2[:], g2[:])
                    nc.sync.dma_start(dbg_xl[sl, :], t2[:])
                    t3 = blk.tile([128, H], F32, tag="dbgc")
                    g3 = gath.tile([128, H], BF16, tag="dbgg")
                    nc.sync.dma_start(g3[:], XRA1[sl, :])
                    nc.vector.tensor_copy(t3[:], g3[:])
                    nc.sync.dma_start(dbg_xra[sl, :], t3[:])
                    t4 = blk.tile([128, L], F32, tag="dbgc2")
                    g4 = gath.tile([128, L], BF16, tag="dbgg2")
                    nc.sync.dma_start(g4[:], XL2_loc[sl, :])
                    nc.vector.tensor_copy(t4[:], g4[:])
                    nc.sync.dma_start(dbg_xl2[sl, :], t4[:])
                    t5 = blk.tile([128, L], F32, tag="dbgc2")
                    g5 = gath.tile([128, L], BF16, tag="dbgg2")
                    nc.sync.dma_start(g5[:], XRA2[sl, :])
                    nc.vector.tensor_copy(t5[:], g5[:])
                    nc.sync.dma_start(dbg_xra2[sl, :], t5[:])
                bnm = cpool.tile([128, 6 * KC1], F32, tag="bnm")
                nc.vector.tensor_copy(bnm[:, 0:KC1], mean[:])
                nc.vector.tensor_copy(bnm[:, KC1:2 * KC1], var[:])
                nc.vector.tensor_copy(bnm[:, 2 * KC1:3 * KC1], scale[:])
                nc.vector.tensor_copy(bnm[:, 3 * KC1:4 * KC1], shift[:])
                nc.vector.tensor_copy(bnm[:, 4 * KC1:5 * KC1], str_sb[:, 0:KC1])
                nc.vector.tensor_copy(bnm[:, 5 * KC1:6 * KC1], str_sb[:, KC1:2 * KC1])
                nc.sync.dma_start(dbg_bn[:, :], bnm[:])
                h1td = cpool.tile([128, NPAD], F32, tag="h1td")
                nc.vector.tensor_copy(h1td[:], h1T[:, 0, :])
                nc.sync.dma_start(dbg_h1t[:, :], h1td[:])
                b2ed = cpool.tile([1, 2 * L], F32, tag="b2ed")
                nc.vector.tensor_copy(b2ed[:1, 0:L], b2e[:1, :])
                nc.vector.tensor_copy(b2ed[:1, L:2 * L], b2era[:1, :])
                nc.sync.dma_start(dbg_b2e[:, :], b2ed[:])

    nc.compile()
    return nc


def _prep_host(inputs):
    x = np.asarray(inputs["x"], dtype=np.float32)
    ei = np.asarray(inputs["edge_index"], dtype=np.int64)
    conv_w = np.asarray(inputs["conv_w"], dtype=np.float32)
    conv_b = np.asarray(inputs["conv_b"], dtype=np.float32)
    W1l = np.asarray(inputs["W1l"], dtype=np.float32)
    b1l = np.asarray(inputs["b1l"], dtype=np.float32)
    W1r = np.asarray(inputs["W1r"], dtype=np.float32)
    b1r = np.asarray(inputs["b1r"], dtype=np.float32)
    att1 = np.asarray(inputs["att1"], dtype=np.float32)
    bias1 = np.asarray(inputs["bias1"], dtype=np.float32)
    gamma = np.asarray(inputs["gamma"], dtype=np.float32)
    beta = np.asarray(inputs["beta"], dtype=np.float32)
    W2l = np.asarray(inputs["W2l"], dtype=np.float32)
    b2l = np.asarray(inputs["b2l"], dtype=np.float32)
    W2r = np.asarray(inputs["W2r"], dtype=np.float32)
    b2r = np.asarray(inputs["b2r"], dtype=np.float32)
    att2 = np.asarray(inputs["att2"], dtype=np.float32)
    bias2 = np.asarray(inputs["bias2"], dtype=np.float32)

    # conv fold: V[(k*32+t), j] = sum_o w[o,k] W[o*32+t, j]
    def fold(W):
        return np.einsum("ok,otj->ktj", conv_w,
                         W.reshape(COUT, TPOS, -1)).reshape(F0, -1)

    V_l, V_r = fold(W1l), fold(W1r)
    be_l = np.einsum("o,otj->j", conv_b, W1l.reshape(COUT, TPOS, H)) + b1l
    be_r = np.einsum("o,otj->j", conv_b, W1r.reshape(COUT, TPOS, H)) + b1r

    perm1 = np.concatenate([np.where(att1 > 0)[0], np.where(att1 <= 0)[0]])
    Fp1 = int((att1 > 0).sum())
    catt1 = np.abs(att1[perm1])
    perm2 = np.concatenate([np.where(att2 > 0)[0], np.where(att2 <= 0)[0]])
    Fp2 = int((att2 > 0).sum())
    catt2 = np.abs(att2[perm2])

    V_lp = V_l[:, perm1]
    be_lp = be_l[perm1]
    V_rap = V_r[:, perm1] * catt1[None, :]
    be_rap = be_r[perm1] * catt1
    bias1_p = bias1[perm1]
    gamma_p, beta_p = gamma[perm1], beta[perm1]
    W2l_p = W2l[perm1][:, perm2]
    W2ra_p = W2r[perm1][:, perm2] * catt2[None, :]
    b2l_p = b2l[perm2]
    b2ra_p = b2r[perm2] * catt2
    bias2_p = bias2[perm2]

    # edges (+ self loops), shard by dst core, sort by dst, block-pad
    loops = np.arange(N, dtype=np.int64)
    src = np.concatenate([ei[0], loops])
    dst = np.concatenate([ei[1], loops])
    per_core = []
    TB = 1
    for c in range(NCORES):
        m = (dst // NLOC) == c
        s_c, d_c = src[m], dst[m] - c * NLOC
        o = np.argsort(d_c, kind="stable")
        s_c, d_c = s_c[o], d_c[o]
        blocks = []
        for b in range(NB):
            bm = (d_c // 128) == b
            blocks.append((s_c[bm], d_c[bm] % 128))
            TB = max(TB, (len(blocks[-1][0]) + 127) // 128)
        per_core.append(blocks)

    core_edges = []
    for c in range(NCORES):
        isrc_a = np.zeros((128, NB, TB), dtype=np.int32)
        idst_a = np.zeros((128, NB, TB), dtype=np.int32)
        drel_a = np.full((128, NB, TB), 300.0, dtype=np.float32)
        for b in range(NB):
            s_b, r_b = per_core[c][b]
            n = len(s_b)
            sg = np.zeros(TB * 128, dtype=np.int32)
            dl = np.zeros(TB * 128, dtype=np.int32)
            dr = np.full(TB * 128, 300.0, dtype=np.float32)
            sg[:n] = s_b.astype(np.int32)
            dl[:n] = (r_b + b * 128).astype(np.int32)
            dr[:n] = r_b.astype(np.float32)
            isrc_a[:, b, :] = sg.reshape(TB, 128).T
            idst_a[:, b, :] = dl.reshape(TB, 128).T
            drel_a[:, b, :] = dr.reshape(TB, 128).T
        core_edges.append((isrc_a, idst_a, drel_a))

    # per-core dense inputs
    flat = x.reshape(N, F0)
    in_maps = []
    for c in range(NCORES):
        fl = np.zeros((NPAD, F0), dtype=np.float32)
        fl[:NLOC] = flat[c * NLOC:(c + 1) * NLOC]
        xT_dev = np.ascontiguousarray(fl.T.reshape(KC0, 128, NPAD).transpose(1, 0, 2))
        isrc_a, idst_a, drel_a = core_edges[c]
        maskl_a = (np.arange(128) < (NLOC - (NB - 1) * 128)).astype(np.float32)[:, None]
        im = {
            "xT": _bf16(xT_dev),
            "Wl": _bf16(V_lp.reshape(KC0, 128, H).transpose(1, 0, 2)),
            "Wra": _bf16(V_rap.reshape(KC0, 128, H).transpose(1, 0, 2)),
            "beff_l": _bf16(be_lp[None, :]),
            "beff_ra": _bf16(be_rap[None, :]),
            "catt1": _bf16(np.tile(catt1, (128, 1))),
            "bias1b": _bf16(np.tile(bias1_p, (128, 1))),
            "W2l": _bf16(W2l_p.reshape(KC1, 128, L).transpose(1, 0, 2)),
            "W2ra": _bf16(W2ra_p.reshape(KC1, 128, L).transpose(1, 0, 2)),
            "b2l": _bf16(b2l_p[None, :]),
            "b2ra": _bf16(b2ra_p[None, :]),
            "catt2": _bf16(np.tile(catt2, (128, 1))),
            "bias2b": np.tile(bias2_p, (128, 1)).astype(np.float32),
            "gamma_c": np.ascontiguousarray(gamma_p.reshape(KC1, 128).T).astype(np.float32),
            "beta_c": np.ascontiguousarray(beta_p.reshape(KC1, 128).T).astype(np.float32),
            "iota_t": np.tile(np.arange(128, dtype=np.float32), (128, 1)).astype(np.float32),
            "maskl": maskl_a.astype(np.float32),
            "isrc": isrc_a,
            "idst": idst_a,
            "dstrel": drel_a.astype(np.float32),
        }
        in_maps.append(im)
    return in_maps, TB, Fp1, Fp2, perm2


LAST_RES = None


def kernel(**inputs):
    global LAST_RES
    in_maps, TB, Fp1, Fp2, perm2 = _prep_host(inputs)
    key = (TB, Fp1, Fp2)
    if key not in _cache:
        _cache[key] = build_program(TB, Fp1, Fp2)
    nc = _cache[key]
    res = run_bass_kernel_spmd(nc, in_maps, list(range(NCORES)))
    LAST_RES = res
    outs = [np.asarray(res.results[c]["mu"], dtype=np.float32)[:NLOC]
            for c in range(NCORES)]
    mu_dev = np.concatenate(outs, axis=0)
    mu = np.empty_like(mu_dev)
    mu[:, perm2] = mu_dev
    return mu



# revision 3
# speedup vs baseline: 91.8701x; 91.8701x over previous
"""Trainium2 Bass kernel for MatrixGATVAE (2-layer GATv2 encoder), 8-core SPMD.

kernel(**inputs): FULL numpy inputs -> FULL [20000, 128] f32 output.
Sharding: nodes + in-edges by destination across 8 cores; weights replicated;
att-weighted source tables all-gathered.

Design (v3):
- Conv1d(kernel==stride) + flatten folded into projection weights on host.
- |att| folded into BOTH projection tables; aggregation un-weights with
  1/|att| after block accumulation, so ONE gathered row per edge serves both
  the attention logits and the message aggregation (no separate xl tables,
  no second/third gather sweep, two all-gathers eliminated vs the original).
- dst-side rows never gathered: z = I @ ga + SelT @ XRA_block composed on the
  TensorEngine into PSUM from an SBUF-resident XRA block, with host-built
  one-hot SelT tiles streamed via sequential (HWDGE) DMA. This halves the
  GpSimd SWDGE descriptor-generation load, which is the serial bottleneck
  for indirect gathers.
- softmax without segment-max (logits are O(1)); BatchNorm folded into
  layer-2 projection weights on device after a tiny AllReduce of the stats.
"""

import math
import sys

import numpy as np

sys.path.insert(0, "/opt/trn_rl_repo")

import concourse.bass as bass
import concourse.bacc as bacc
import concourse.mybir as mybir
import concourse.tile as tile
from concourse.bass_utils import run_bass_kernel_spmd

BF16 = mybir.dt.bfloat16
F32 = mybir.dt.float32
I32 = mybir.dt.int32
AF = mybir.ActivationFunctionType
ALU = mybir.AluOpType

N = 20000
NCORES = 8
NLOC = N // NCORES            # 2500
NPAD = 2560
NB = NPAD // 128              # 20
KW, TPOS, COUT = 24, 32, 64
F0 = 768
H = 512
L = 128
KC0 = F0 // 128               # 6
KC1 = H // 128                # 4
BN_EPS = 1e-5
ALPHA = 0.2

_cache = {}


def _bf16(a):
    import ml_dtypes
    return np.ascontiguousarray(np.asarray(a, dtype=np.float32)).astype(ml_dtypes.bfloat16)


def build_program(TB, Fp1, Fp2, reps=1):
    nc = bacc.Bacc(num_devices=NCORES)

    def dp(name, shape, dtype, isOutput=False):
        return nc.declare_dram_parameter(name, shape, dtype, isOutput)

    xT = dp("xT", [128, KC0, NPAD], BF16)
    Wla = dp("Wla", [128, KC0, H], BF16)
    Wra = dp("Wra", [128, KC0, H], BF16)
    beff_la = dp("beff_la", [1, H], BF16)
    beff_ra = dp("beff_ra", [1, H], BF16)
    invc1 = dp("invc1", [128, H], F32)
    bias1b = dp("bias1b", [128, H], BF16)
    W2la = dp("W2la", [128, KC1, L], BF16)
    W2ra = dp("W2ra", [128, KC1, L], BF16)
    b2la = dp("b2la", [1, L], BF16)
    b2ra = dp("b2ra", [1, L], BF16)
    invc2 = dp("invc2", [128, L], F32)
    bias2b = dp("bias2b", [128, L], F32)
    gamma_c = dp("gamma_c", [128, KC1], F32)
    beta_c = dp("beta_c", [128, KC1], F32)
    iota_t = dp("iota_t", [128, 128], F32)
    maskl = dp("maskl", [128, 1], F32)
    isrc = dp("isrc", [128, NB, TB], I32)
    dstrel = dp("dstrel", [128, NB, TB], F32)
    selt = dp("selt", [128, NB, TB * 128], BF16)
    eye = dp("eye", [128, 128], BF16)
    mu_out = dp("mu", [NPAD, L], F32, isOutput=True)

    XLA_loc = nc.dram_tensor("XLA_loc", [NPAD, H], BF16)
    XLA = nc.dram_tensor("XLA", [N, H], BF16, addr_space="Shared")
    H1Dc = [nc.dram_tensor(f"H1Dc{c}", [NPAD, 128], BF16) for c in range(KC1)]
    XLA2_loc = nc.dram_tensor("XLA2_loc", [NPAD, L], BF16)
    XLA2 = nc.dram_tensor("XLA2", [N, L], BF16, addr_space="Shared")
    ST_loc = nc.dram_tensor("ST_loc", [128, 2 * KC1], F32)
    ST_red = nc.dram_tensor("ST_red", [128, 2 * KC1], F32, addr_space="Shared")

    grp = [list(range(NCORES))]

    with tile.TileContext(nc) as tc:
      for rep in range(reps):
        with (
            tc.tile_pool(name=f"const{rep}", bufs=1) as cpool,
            tc.tile_pool(name=f"work{rep}", bufs=3) as work,
            tc.tile_pool(name=f"gath{rep}", bufs=3) as gath,
            tc.tile_pool(name=f"gblk{rep}", bufs=2) as gblk,
            tc.tile_pool(name=f"blk{rep}", bufs=2) as blk,
            tc.tile_pool(name=f"acc{rep}", bufs=2, space="PSUM") as ps,
            tc.tile_pool(name=f"psZ{rep}", bufs=2, space="PSUM") as ps_z,
            tc.tile_pool(name=f"psS{rep}", bufs=2, space="PSUM") as ps_s,
            tc.tile_pool(name=f"psT{rep}", bufs=2, space="PSUM") as ps_st,
        ):
            def load(tag, dram):
                t = cpool.tile(dram.shape, dram.dtype, tag=tag)
                nc.sync.dma_start(t[:], dram[:])
                return t

            xT_s = load("xT", xT)
            Wla_s = load("Wla", Wla)
            Wra_s = load("Wra", Wra)
            beffla_s = load("beffla", beff_la)
            beffra_s = load("beffra", beff_ra)
            invc1_s = load("invc1", invc1)
            bias1b_s = load("bias1b", bias1b)
            W2la_s = load("W2la", W2la)
            W2ra_s = load("W2ra", W2ra)
            b2la_s = load("b2la", b2la)
            b2ra_s = load("b2ra", b2ra)
            invc2_s = load("invc2", invc2)
            bias2b_s = load("bias2b", bias2b)
            gamma_s = load("gamma", gamma_c)
            beta_s = load("beta", beta_c)
            iota_s = load("iota", iota_t)
            maskl_s = load("maskl", maskl)
            isrc_s = load("isrc", isrc)
            dstrel_s = load("dstrel", dstrel)
            eye_s = load("eye", eye)
            XRAres1 = cpool.tile([128, NB, H], BF16, tag="XRAres1")
            XRAres2 = cpool.tile([128, NB, L], BF16, tag="XRAres2")

            ones_col = cpool.tile([128, 1], BF16, tag="ones_col")
            nc.vector.memset(ones_col[:], 1.0)
            one_row = cpool.tile([1, 128], BF16, tag="one_row")
            nc.vector.memset(one_row[:], 1.0)

            Q1 = cpool.tile([128, NB, TB], F32, tag="Q1")
            Q2 = cpool.tile([128, NB, TB], F32, tag="Q2")
            AA = cpool.tile([128, NB, TB], F32, tag="AA")
            h1T = cpool.tile([128, KC1, NPAD], BF16, tag="h1T")
            W2fl = cpool.tile([128, KC1, L], BF16, tag="W2fl")
            W2fra = cpool.tile([128, KC1, L], BF16, tag="W2fra")
            b2e = cpool.tile([1, L], BF16, tag="b2e")
            b2era = cpool.tile([1, L], BF16, tag="b2era")

            # ============ layer-1 projections -> tables ============
            for b in range(NB):
                sl = slice(b * 128, (b + 1) * 128)
                pl = ps.tile([128, H], F32, tag="acc")
                pra = ps.tile([128, H], F32, tag="acc")
                for c in range(KC0):
                    lhsT = xT_s[:, c, sl]
                    nc.tensor.matmul(pl[:], lhsT, Wla_s[:, c, :], start=(c == 0), stop=False)
                    nc.tensor.matmul(pra[:], lhsT, Wra_s[:, c, :], start=(c == 0), stop=False)
                nc.tensor.matmul(pl[:], one_row[:1, :], beffla_s[:1, :], start=False, stop=True)
                nc.tensor.matmul(pra[:], one_row[:1, :], beffra_s[:1, :], start=False, stop=True)
                xla_sb = blk.tile([128, H], BF16, tag="xla")
                nc.vector.tensor_copy(xla_sb[:], pl[:])
                nc.scalar.activation(XRAres1[:, b, :], pra[:], AF.Copy)
                nc.sync.dma_start(XLA_loc[sl, :], xla_sb[:])

            nc.gpsimd.collective_compute(
                "AllGather", ALU.bypass, replica_groups=grp,
                ins=[XLA_loc[0:NLOC, :]], outs=[XLA[:, :]])

            st_acc = cpool.tile([128, 2 * KC1], F32, tag="st_acc")
            nc.vector.memset(st_acc[:], 0.0)

            # ============ fused edge stage ============
            def edge_stage(W, Fp, TBLA, XRAres, lay, post):
                for b in range(NB):
                    gaB = gblk.tile([128, TB, W], BF16, tag=f"gaB{lay}")
                    selt_b = gblk.tile([128, TB * 128], BF16, tag=f"selt{lay}")
                    nc.sync.dma_start(selt_b[:], selt[:, b, :])
                    # pass 1: gather + PE-composed z + logits
                    for t in range(TB):
                        nc.gpsimd.indirect_dma_start(
                            out=gaB[:, t, :], out_offset=None, in_=TBLA[:, :],
                            in_offset=bass.IndirectOffsetOnAxis(
                                ap=isrc_s[:, b, t:t + 1], axis=0))
                        pz = ps_z.tile([128, W], F32, tag="pz")
                        nc.tensor.matmul(pz[:], eye_s[:], gaB[:, t, :],
                                         start=True, stop=False)
                        nc.tensor.matmul(pz[:], selt_b[:, t * 128:(t + 1) * 128],
                                         XRAres[:, b, :], start=False, stop=True)
                        lr = work.tile([128, W], BF16, tag=f"lr{lay}")
                        if Fp > 0:
                            nc.scalar.activation(
                                lr[:, 0:Fp], pz[:, 0:Fp], AF.Prelu,
                                alpha=ALPHA, accum_out=Q1[:, b, t:t + 1])
                        else:
                            nc.vector.memset(Q1[:, b, t:t + 1], 0.0)
                        if Fp < W:
                            nc.scalar.activation(
                                lr[:, Fp:W], pz[:, Fp:W], AF.Prelu,
                                alpha=ALPHA, accum_out=Q2[:, b, t:t + 1])
                        else:
                            nc.vector.memset(Q2[:, b, t:t + 1], 0.0)
                    # softmax numerators for this block
                    nc.vector.tensor_tensor(
                        AA[:, b, :], Q1[:, b, :], Q2[:, b, :], ALU.subtract)
                    nc.scalar.activation(AA[:, b, :], AA[:, b, :], AF.Exp)
                    # pass 2: aggregate (reuses gaB)
                    pU = ps.tile([128, W], F32, tag="acc")
                    pS = ps_s.tile([128, 1], F32, tag="pS")
                    for t in range(TB):
                        Sa = work.tile([128, 128], BF16, tag=f"Sa{lay}")
                        nc.vector.tensor_scalar(
                            Sa[:], iota_s[:], dstrel_s[:, b, t:t + 1],
                            AA[:, b, t:t + 1], ALU.is_equal, ALU.mult)
                        nc.tensor.matmul(
                            pU[:], Sa[:], gaB[:, t, :],
                            start=(t == 0), stop=(t == TB - 1))
                        nc.tensor.matmul(
                            pS[:], Sa[:], ones_col[:],
                            start=(t == 0), stop=(t == TB - 1))
                    post(b, pU, pS)

            # ---- layer-1 fused edge stage -> h1 ----
            def post1(b, pU, pS):
                s_sb = blk.tile([128, 1], F32, tag="s1")
                nc.vector.tensor_scalar(s_sb[:], pS[:], 1e-16, None, ALU.add)
                r_sb = blk.tile([128, 1], F32, tag="r1")
                nc.vector.reciprocal(r_sb[:], s_sb[:])
                u1 = blk.tile([128, H], F32, tag="u1")
                nc.vector.tensor_scalar(u1[:], pU[:], r_sb[:], None, ALU.mult)
                h1 = blk.tile([128, H], BF16, tag="h1")
                nc.vector.tensor_tensor(h1[:], u1[:], invc1_s[:], ALU.mult)
                nc.vector.tensor_tensor(h1[:], h1[:], bias1b_s[:], ALU.add)
                nc.vector.tensor_scalar(h1[:], h1[:], 0.0, None, ALU.max)
                if b == NB - 1:
                    nc.vector.tensor_scalar(h1[:], h1[:], maskl_s[:], None, ALU.mult)
                sq = blk.tile([128, H], BF16, tag="sq")
                nc.scalar.activation(sq[:], h1[:], AF.Square)
                pmb = ps_st.tile([128, 2 * KC1], F32, tag="st")
                for c in range(KC1):
                    nc.tensor.matmul(
                        pmb[:, c:c + 1], h1[:, c * 128:(c + 1) * 128], ones_col[:],
                        start=True, stop=True)
                    nc.tensor.matmul(
                        pmb[:, KC1 + c:KC1 + c + 1], sq[:, c * 128:(c + 1) * 128], ones_col[:],
                        start=True, stop=True)
                nc.vector.tensor_tensor(st_acc[:], st_acc[:], pmb[:], ALU.add)
                for c in range(KC1):
                    nc.sync.dma_start(
                        H1Dc[c][b * 128:(b + 1) * 128, :],
                        h1[:, c * 128:(c + 1) * 128])

            edge_stage(H, Fp1, XLA, XRAres1, 1, post1)

            # ---- BN stats all-reduce + fold into layer-2 weights ----
            nc.sync.dma_start(ST_loc[:, :], st_acc[:])
            nc.gpsimd.collective_compute(
                "AllReduce", ALU.add, replica_groups=grp,
                ins=[ST_loc[:, :]], outs=[ST_red[:, :]])
            str_sb = cpool.tile([128, 2 * KC1], F32, tag="str_sb")
            nc.sync.dma_start(str_sb[:], ST_red[:, :])

            mean = cpool.tile([128, KC1], F32, tag="mean")
            var = cpool.tile([128, KC1], F32, tag="var")
            nc.vector.tensor_scalar(mean[:], str_sb[:, 0:KC1], 1.0 / N, None, ALU.mult)
            nc.vector.tensor_scalar(var[:], str_sb[:, KC1:2 * KC1], 1.0 / N, None, ALU.mult)
            m2 = cpool.tile([128, KC1], F32, tag="m2")
            nc.vector.tensor_tensor(m2[:], mean[:], mean[:], ALU.mult)
            nc.vector.tensor_tensor(var[:], var[:], m2[:], ALU.subtract)
            nc.vector.tensor_scalar(var[:], var[:], BN_EPS, None, ALU.add)
            sd = cpool.tile([128, KC1], F32, tag="sd")
            nc.scalar.activation(sd[:], var[:], AF.Sqrt)
            rsd = cpool.tile([128, KC1], F32, tag="rsd")
            nc.vector.reciprocal(rsd[:], sd[:])
            tmpn = cpool.tile([128, KC1], F32, tag="tmpn")
            nc.vector.tensor_tensor(tmpn[:], var[:], rsd[:], ALU.mult)
            nc.vector.tensor_tensor(sd[:], sd[:], tmpn[:], ALU.add)
            nc.vector.tensor_scalar(sd[:], sd[:], 0.5, None, ALU.mult)
            nc.vector.reciprocal(rsd[:], sd[:])
            scale = cpool.tile([128, KC1], F32, tag="scale")
            nc.vector.tensor_tensor(scale[:], gamma_s[:], rsd[:], ALU.mult)
            shift = cpool.tile([128, KC1], F32, tag="shift")
            nc.vector.tensor_tensor(shift[:], mean[:], scale[:], ALU.mult)
            nc.vector.tensor_tensor(shift[:], beta_s[:], shift[:], ALU.subtract)
            shift_bf = cpool.tile([128, KC1], BF16, tag="shift_bf")
            nc.vector.tensor_copy(shift_bf[:], shift[:])

            for c in range(KC1):
                nc.vector.tensor_scalar(
                    W2fl[:, c, :], W2la_s[:, c, :], scale[:, c:c + 1], None, ALU.mult)
                nc.vector.tensor_scalar(
                    W2fra[:, c, :], W2ra_s[:, c, :], scale[:, c:c + 1], None, ALU.mult)
            pb = ps_st.tile([1, L], F32, tag="st")
            pbra = ps_st.tile([1, L], F32, tag="st")
            for c in range(KC1):
                nc.tensor.matmul(pb[:1, :], shift_bf[:, c:c + 1], W2la_s[:, c, :],
                                 start=(c == 0), stop=False)
                nc.tensor.matmul(pbra[:1, :], shift_bf[:, c:c + 1], W2ra_s[:, c, :],
                                 start=(c == 0), stop=False)
            nc.tensor.matmul(pb[:1, :], one_row[:1, 0:1], b2la_s[:1, :], start=False, stop=True)
            nc.tensor.matmul(pbra[:1, :], one_row[:1, 0:1], b2ra_s[:1, :], start=False, stop=True)
            nc.vector.tensor_copy(b2e[:], pb[:1, :])
            nc.vector.tensor_copy(b2era[:], pbra[:1, :])

            # h1T via DMA transpose (bf16, part%16, free%128)
            for c in range(KC1):
                nc.sync.dma_start(h1T[:, c, :], H1Dc[c][:, :], transpose=True)

            # ---- layer-2 projections -> tables ----
            for b in range(NB):
                sl = slice(b * 128, (b + 1) * 128)
                p2 = ps.tile([128, L], F32, tag="acc")
                p2ra = ps.tile([128, L], F32, tag="acc")
                for c in range(KC1):
                    lhsT = h1T[:, c, sl]
                    nc.tensor.matmul(p2[:], lhsT, W2fl[:, c, :], start=(c == 0), stop=False)
                    nc.tensor.matmul(p2ra[:], lhsT, W2fra[:, c, :], start=(c == 0), stop=False)
                nc.tensor.matmul(p2[:], one_row[:1, :], b2e[:1, :], start=False, stop=True)
                nc.tensor.matmul(p2ra[:], one_row[:1, :], b2era[:1, :], start=False, stop=True)
                xla2_sb = blk.tile([128, L], BF16, tag="xla2")
                nc.vector.tensor_copy(xla2_sb[:], p2[:])
                nc.scalar.activation(XRAres2[:, b, :], p2ra[:], AF.Copy)
                nc.sync.dma_start(XLA2_loc[sl, :], xla2_sb[:])

            nc.gpsimd.collective_compute(
                "AllGather", ALU.bypass, replica_groups=grp,
                ins=[XLA2_loc[0:NLOC, :]], outs=[XLA2[:, :]])

            # ---- layer-2 fused edge stage -> mu ----
            def post2(b, pU, pS):
                s_sb = blk.tile([128, 1], F32, tag="s2")
                nc.vector.tensor_scalar(s_sb[:], pS[:], 1e-16, None, ALU.add)
                r_sb = blk.tile([128, 1], F32, tag="r2")
                nc.vector.reciprocal(r_sb[:], s_sb[:])
                u2 = blk.tile([128, L], F32, tag="u2")
                nc.vector.tensor_scalar(u2[:], pU[:], r_sb[:], None, ALU.mult)
                mu_sb = blk.tile([128, L], F32, tag="mu")
                nc.vector.tensor_tensor(mu_sb[:], u2[:], invc2_s[:], ALU.mult)
                nc.vector.tensor_tensor(mu_sb[:], mu_sb[:], bias2b_s[:], ALU.add)
                nc.sync.dma_start(mu_out[b * 128:(b + 1) * 128, :], mu_sb[:])

            edge_stage(L, Fp2, XLA2, XRAres2, 2, post2)

    nc.compile()
    return nc


def _prep_host(inputs):
    x = np.asarray(inputs["x"], dtype=np.float32)
    ei = np.asarray(inputs["edge_index"], dtype=np.int64)
    conv_w = np.asarray(inputs["conv_w"], dtype=np.float32)
    conv_b = np.asarray(inputs["conv_b"], dtype=np.float32)
    W1l = np.asarray(inputs["W1l"], dtype=np.float32)
    b1l = np.asarray(inputs["b1l"], dtype=np.float32)
    W1r = np.asarray(inputs["W1r"], dtype=np.float32)
    b1r = np.asarray(inputs["b1r"], dtype=np.float32)
    att1 = np.asarray(inputs["att1"], dtype=np.float32)
    bias1 = np.asarray(inputs["bias1"], dtype=np.float32)
    gamma = np.asarray(inputs["gamma"], dtype=np.float32)
    beta = np.asarray(inputs["beta"], dtype=np.float32)
    W2l = np.asarray(inputs["W2l"], dtype=np.float32)
    b2l = np.asarray(inputs["b2l"], dtype=np.float32)
    W2r = np.asarray(inputs["W2r"], dtype=np.float32)
    b2r = np.asarray(inputs["b2r"], dtype=np.float32)
    att2 = np.asarray(inputs["att2"], dtype=np.float32)
    bias2 = np.asarray(inputs["bias2"], dtype=np.float32)

    # conv fold: V[(k*32+t), j] = sum_o w[o,k] W[o*32+t, j]
    def fold(W):
        return np.einsum("ok,otj->ktj", conv_w,
                         W.reshape(COUT, TPOS, -1)).reshape(F0, -1)

    V_l, V_r = fold(W1l), fold(W1r)
    be_l = np.einsum("o,otj->j", conv_b, W1l.reshape(COUT, TPOS, H)) + b1l
    be_r = np.einsum("o,otj->j", conv_b, W1r.reshape(COUT, TPOS, H)) + b1r

    perm1 = np.concatenate([np.where(att1 > 0)[0], np.where(att1 <= 0)[0]])
    Fp1 = int((att1 > 0).sum())
    catt1 = np.abs(att1[perm1])
    catt1g = np.maximum(catt1, 1e-20)
    perm2 = np.concatenate([np.where(att2 > 0)[0], np.where(att2 <= 0)[0]])
    Fp2 = int((att2 > 0).sum())
    catt2 = np.abs(att2[perm2])
    catt2g = np.maximum(catt2, 1e-20)

    # att magnitude folded into BOTH tables (src side att-weighted; recover
    # un-weighted aggregation by multiplying with 1/|att| after accumulation)
    V_lap = V_l[:, perm1] * catt1g[None, :]
    be_lap = be_l[perm1] * catt1g
    V_rap = V_r[:, perm1] * catt1g[None, :]
    be_rap = be_r[perm1] * catt1g
    bias1_p = bias1[perm1]
    gamma_p, beta_p = gamma[perm1], beta[perm1]
    W2la_p = W2l[perm1][:, perm2] * catt2g[None, :]
    W2ra_p = W2r[perm1][:, perm2] * catt2g[None, :]
    b2la_p = b2l[perm2] * catt2g
    b2ra_p = b2r[perm2] * catt2g
    bias2_p = bias2[perm2]

    # edges (+ self loops), shard by dst core, sort by dst, block-pad
    loops = np.arange(N, dtype=np.int64)
    src = np.concatenate([ei[0], loops])
    dst = np.concatenate([ei[1], loops])
    per_core = []
    TB = 1
    for c in range(NCORES):
        m = (dst // NLOC) == c
        s_c, d_c = src[m], dst[m] - c * NLOC
        o = np.argsort(d_c, kind="stable")
        s_c, d_c = s_c[o], d_c[o]
        blocks = []
        for b in range(NB):
            bm = (d_c // 128) == b
            blocks.append((s_c[bm], d_c[bm] % 128))
            TB = max(TB, (len(blocks[-1][0]) + 127) // 128)
        per_core.append(blocks)

    core_edges = []
    for c in range(NCORES):
        isrc_a = np.zeros((128, NB, TB), dtype=np.int32)
        idst_a = np.zeros((128, NB, TB), dtype=np.int32)
        drel_a = np.full((128, NB, TB), 300.0, dtype=np.float32)
        for b in range(NB):
            s_b, r_b = per_core[c][b]
            n = len(s_b)
            sg = np.zeros(TB * 128, dtype=np.int32)
            dl = np.zeros(TB * 128, dtype=np.int32)
            dr = np.full(TB * 128, 300.0, dtype=np.float32)
            sg[:n] = s_b.astype(np.int32)
            dl[:n] = (r_b + b * 128).astype(np.int32)
            dr[:n] = r_b.astype(np.float32)
            isrc_a[:, b, :] = sg.reshape(TB, 128).T
            idst_a[:, b, :] = dl.reshape(TB, 128).T
            drel_a[:, b, :] = dr.reshape(TB, 128).T
        core_edges.append((isrc_a, idst_a, drel_a))

    # per-core dense inputs
    flat = x.reshape(N, F0)
    in_maps = []
    for c in range(NCORES):
        fl = np.zeros((NPAD, F0), dtype=np.float32)
        fl[:NLOC] = flat[c * NLOC:(c + 1) * NLOC]
        xT_dev = np.ascontiguousarray(fl.T.reshape(KC0, 128, NPAD).transpose(1, 0, 2))
        isrc_a, idst_a, drel_a = core_edges[c]
        maskl_a = (np.arange(128) < (NLOC - (NB - 1) * 128)).astype(np.float32)[:, None]
        seltT = (drel_a.transpose(1, 2, 0)[None, :, :, :]
                 == np.arange(128, dtype=np.float32)[:, None, None, None])
        selt_host = seltT.astype(np.float32).reshape(128, NB, TB * 128)
        im = {
            "xT": _bf16(xT_dev),
            "selt": _bf16(selt_host),
            "eye": _bf16(np.eye(128, dtype=np.float32)),
            "Wla": _bf16(V_lap.reshape(KC0, 128, H).transpose(1, 0, 2)),
            "Wra": _bf16(V_rap.reshape(KC0, 128, H).transpose(1, 0, 2)),
            "beff_la": _bf16(be_lap[None, :]),
            "beff_ra": _bf16(be_rap[None, :]),
            "invc1": np.tile(1.0 / catt1g, (128, 1)).astype(np.float32),
            "bias1b": _bf16(np.tile(bias1_p, (128, 1))),
            "W2la": _bf16(W2la_p.reshape(KC1, 128, L).transpose(1, 0, 2)),
            "W2ra": _bf16(W2ra_p.reshape(KC1, 128, L).transpose(1, 0, 2)),
            "b2la": _bf16(b2la_p[None, :]),
            "b2ra": _bf16(b2ra_p[None, :]),
            "invc2": np.tile(1.0 / catt2g, (128, 1)).astype(np.float32),
            "bias2b": np.tile(bias2_p, (128, 1)).astype(np.float32),
            "gamma_c": np.ascontiguousarray(gamma_p.reshape(KC1, 128).T).astype(np.float32),
            "beta_c": np.ascontiguousarray(beta_p.reshape(KC1, 128).T).astype(np.float32),
            "iota_t": np.tile(np.arange(128, dtype=np.float32), (128, 1)).astype(np.float32),
            "maskl": maskl_a.astype(np.float32),
            "isrc": isrc_a,
            "dstrel": drel_a.astype(np.float32),
        }
        in_maps.append(im)
    return in_maps, TB, Fp1, Fp2, perm2


LAST_RES = None


def kernel(**inputs):
    global LAST_RES
    in_maps, TB, Fp1, Fp2, perm2 = _prep_host(inputs)
    key = (TB, Fp1, Fp2)
    if key not in _cache:
        _cache[key] = build_program(TB, Fp1, Fp2)
    nc = _cache[key]
    res = run_bass_kernel_spmd(nc, in_maps, list(range(NCORES)))
    LAST_RES = res
    outs = [np.asarray(res.results[c]["mu"], dtype=np.float32)[:NLOC]
            for c in range(NCORES)]
    mu_dev = np.concatenate(outs, axis=0)
    mu = np.empty_like(mu_dev)
    mu[:, perm2] = mu_dev
    return mu
